# revision 26
# baseline (speedup 1.0000x reference)
"""Trainium2 Bass kernel for nn_AttentionBlock (dense transformer block).

Sharding: 8 cores = (2 batches x 4 seq-chunks of 512 tokens).
Each core: rmsnorm + QKV + rope for its 512-token chunk, chunked AllGather of
K/V (fp8) across its batch's 4 cores, full attention for its queries over all
16 heads, then wo + residual + rmsnorm + SwiGLU FFN for its rows.

dtypes: fp8e4m3 for QKV/wo weights+activations (DoubleRow matmuls), fp8 for
K/V gather, scores, probs and PV; bf16 for the FFN.  Scales: xn is written as
64*xn (folded into the rmsnorm broadcast), weights quantized at scale 1.0,
probs = 16*exp(score) via the exp bias; all scales cancel in the softmax
normalization except a single 1/64 on the wo PSUM evacuation.
"""

from contextlib import ExitStack

import numpy as np
import ml_dtypes

import concourse.bacc as bacc
import concourse.mybir as mybir
import concourse.tile as tile
from concourse.bass_utils import run_bass_kernel_spmd

DIM = 1024
NHEAD = 16
HD = 64
SEQ = 2048
BSZ = 2
FFN = 2816
EPS = 1e-6
NCORES = 8
GROUP = 4             # cores per batch (sequence-sharding group)
CHUNK = SEQ // GROUP  # 512 local tokens per core
NKT = SEQ // 128      # 16 key tiles
NFT = DIM // 128      # 8 feature tiles
NMID = FFN // 128     # 22 ffn hidden tiles
NK2 = DIM // 256      # 4 doublerow contraction steps over DIM

# NOTE: hw float8e4 is IEEE e4m3 (max finite 240, has inf) - keep scaled
# values comfortably below 240.
XSCALE = 32.0         # fp8 scale on normalized activations
ESCALE = 0.125 / (XSCALE * XSCALE)  # score descale folded into exp

FP32 = mybir.dt.float32
F32R = mybir.dt.float32r
BF16 = mybir.dt.bfloat16
FP8 = mybir.dt.float8e4
AF = mybir.ActivationFunctionType
DR = mybir.MatmulPerfMode.DoubleRow
BF16_NP = ml_dtypes.bfloat16
FP8_NP = ml_dtypes.float8_e4m3

_prog_cache = {}


def _build_program(use_mask: bool, apply_attn_w: bool, apply_ffn_w: bool,
                   debug: bool = False):
    nc = bacc.Bacc("TRN2", target_bir_lowering=False, debug=False,
                   num_devices=NCORES)

    # ---- DRAM I/O ----
    x_fm = nc.dram_tensor("x_fm", (DIM, CHUNK), FP32, kind="ExternalInput").ap()
    csa = nc.dram_tensor("csa", (128, CHUNK), FP32, kind="ExternalInput").ap()
    csb = nc.dram_tensor("csb", (128, CHUNK), FP32, kind="ExternalInput").ap()
    ones_col = nc.dram_tensor("ones_col", (128, 1), F32R, kind="ExternalInput").ap()
    ones_row = nc.dram_tensor("ones_row", (1, 128), FP32, kind="ExternalInput").ap()
    row64 = nc.dram_tensor("row64", (1, 128), FP32, kind="ExternalInput").ap()
    e2 = nc.dram_tensor("e2", (2, 128), FP32, kind="ExternalInput").ap()
    # qkv/wo weights: doublerow-tiled [4*128, 2*1024] fp8 (host-tiled)
    wqt = nc.dram_tensor("wqt", (NK2 * 128, 2 * DIM), FP8, kind="ExternalInput").ap()
    wkt = nc.dram_tensor("wkt", (NK2 * 128, 2 * DIM), FP8, kind="ExternalInput").ap()
    wvt = nc.dram_tensor("wvt", (NK2 * 128, 2 * DIM), FP8, kind="ExternalInput").ap()
    wot = nc.dram_tensor("wot", (NK2 * 128, 2 * DIM), FP8, kind="ExternalInput").ap()
    # w1t/w3t: pre-tiled [f, p, (k c)]; w2t: pre-tiled [m, p, (k c)] (host)
    w1t = nc.dram_tensor("w1t", (FFN, DIM), BF16, kind="ExternalInput").ap()
    w3t = nc.dram_tensor("w3t", (FFN, DIM), BF16, kind="ExternalInput").ap()
    w2t = nc.dram_tensor("w2t", (DIM, FFN), BF16, kind="ExternalInput").ap()
    if apply_attn_w:
        attnw = nc.dram_tensor("attnw", (DIM, 1), FP32, kind="ExternalInput").ap()
    if apply_ffn_w:
        ffnw = nc.dram_tensor("ffnw", (DIM, 1), FP32, kind="ExternalInput").ap()
    if use_mask:
        maskt = nc.dram_tensor("maskt", (SEQ, CHUNK), FP32, kind="ExternalInput").ap()
    out_fm = nc.dram_tensor("out_fm", (DIM, CHUNK), FP32, kind="ExternalOutput").ap()
    dbg = {}
    if debug:
        for nm, shape, ddt in [
                ("dbg_xn", (4 * 128, 2 * CHUNK), FP8),
                ("dbg_q", (DIM, CHUNK), FP8),
                ("dbg_kga", (GROUP * 512, CHUNK), FP8),
                ("dbg_vga", (GROUP * CHUNK, 512), FP8),
                ("dbg_p0", (128, NKT * 512), FP8),
                ("dbg_attn", (4 * 128, 2 * CHUNK), FP8),
                ("dbg_h", (DIM, CHUNK), FP32),
                ("dbg_hn", (DIM, CHUNK), BF16)]:
            dbg[nm] = nc.dram_tensor(nm, shape, ddt, kind="ExternalOutput").ap()

    groups = [list(range(GROUP)), list(range(GROUP, 2 * GROUP))]

    with tile.TileContext(nc) as tc, ExitStack() as ctx:
        # long-lived pool: constants + tiles that cross phase boundaries
        p0 = ctx.enter_context(tc.tile_pool(name="p0", bufs=1))
        dr = ctx.enter_context(tc.tile_pool(name="dr", bufs=1, space="DRAM"))
        # pool for tiles only needed through phase B (freed before the FFN)
        ctxAB = ExitStack()
        pAB = ctxAB.enter_context(tc.tile_pool(name="pAB", bufs=1))

        # fire the CC-stream warmup collective first: its ~30us firmware
        # startup runs while the DMAs and phase A compute proceed
        warm_in = dr.tile([128, 4], BF16, name="warm_in")
        warm_out = dr.tile([GROUP * 128, 4], BF16, name="warm_out")
        warm_sb = pAB.tile([128, 4], BF16, name="warm_sb")
        nc.vector.memset(warm_sb[:], 0.0)
        nc.sync.dma_start(warm_in[:], warm_sb[:])
        nc.gpsimd.collective_compute(
            "AllGather", mybir.AluOpType.bypass, replica_groups=groups,
            ins=[warm_in.opt()], outs=[warm_out.opt()])

        cos_t = pAB.tile([128, CHUNK], FP32, name="cos_t")
        sin_t = pAB.tile([128, CHUNK], FP32, name="sin_t")
        onesc_t = p0.tile([128, 1], F32R, name="onesc_t")
        onesr_t = p0.tile([1, 128], FP32, name="onesr_t")
        row64_t = p0.tile([1, 128], FP32, name="row64_t")
        e2_t = p0.tile([2, 128], FP32, name="e2_t")
        eps_t = p0.tile([1, 1], FP32, name="eps_t")
        nc.vector.memset(eps_t[:], EPS)
        nc.sync.dma_start(cos_t[:], csa[:])
        nc.sync.dma_start(sin_t[:], csb[:])
        nc.sync.dma_start(onesc_t[:], ones_col[:])
        nc.sync.dma_start(onesr_t[:], ones_row[:])
        nc.sync.dma_start(row64_t[:], row64[:])
        nc.sync.dma_start(e2_t[:], e2[:])
        attnw_t = ffnw_t = None
        if apply_attn_w:
            attnw_t = p0.tile([128, NFT], FP32, name="attnw_t")
            nc.sync.dma_start(
                attnw_t[:], attnw.rearrange("(k p) o -> p (k o)", p=128))
        if apply_ffn_w:
            ffnw_t = p0.tile([128, NFT], FP32, name="ffnw_t")
            nc.sync.dma_start(
                ffnw_t[:], ffnw.rearrange("(k p) o -> p (k o)", p=128))

        # PE warm-up: a short dense bf16 matmul burst so HAM reaches 8/8
        # before the real pipeline starts
        warm_bf = pAB.tile([128, CHUNK], BF16, name="warm_bf")
        nc.vector.tensor_copy(warm_bf[:], cos_t[:])
        with tc.tile_pool(name="psW", bufs=1, space="PSUM") as psW:
            wu_ps = psW.tile([128, 512], FP32, name="wu_ps", tag="wu")
            for it in range(14):
                nc.tensor.matmul(wu_ps[:], warm_bf[:, 0:128], warm_bf[:],
                                 start=(it == 0), stop=(it == 13))

        # hidden chunk (feature-major, kept for the attention residual)
        x_t = []
        for i in range(NFT):
            t = p0.tile([128, CHUNK], FP32, name=f"x_{i}", tag="x", bufs=NFT)
            nc.sync.dma_start(t[:], x_fm[i * 128:(i + 1) * 128, :])
            x_t.append(t)

        # wo weights (doublerow fp8): load early, consumed in phase C
        wo_sb = []
        for k in range(NK2):
            t = p0.tile([128, 2 * DIM], FP8, name=f"wo_{k}", tag="wo", bufs=NK2)
            nc.sync.dma_start(t[:], wot[k * 128:(k + 1) * 128, :])
            wo_sb.append(t)

        def rmsnorm_stats(sb, ps, src_tiles, tagp):
            """src (8 fm fp32 tiles) -> [1, CHUNK] reciprocal rms (fp32)."""
            ss_ps = ps.tile([1, CHUNK], FP32, name=f"ss_{tagp}", tag="ss")
            for i in range(NFT):
                sq = sb.tile([128, CHUNK], F32R, name=f"sq_{tagp}_{i}",
                             tag="sq", bufs=4)
                if i % 2 == 0:
                    nc.scalar.activation(sq[:], src_tiles[i][:], AF.Square)
                else:
                    nc.vector.tensor_mul(sq[:], src_tiles[i][:],
                                         src_tiles[i][:])
                nc.tensor.matmul(ss_ps[:], onesc_t[:], sq[:],
                                 start=(i == 0), stop=(i == NFT - 1))
            # v = mean + eps ; r = rsqrt(v) with one Newton step
            v_t = sb.tile([1, CHUNK], FP32, name=f"v_{tagp}", tag="nv")
            nc.scalar.activation(v_t[:], ss_ps[:], AF.Copy,
                                 scale=1.0 / DIM, bias=EPS)
            sd_t = sb.tile([1, CHUNK], FP32, name=f"sd_{tagp}", tag="nsd")
            nc.scalar.activation(sd_t[:], ss_ps[:], AF.Sqrt,
                                 scale=1.0 / DIM, bias=eps_t[:])
            r0_t = sb.tile([1, CHUNK], FP32, name=f"r0_{tagp}", tag="nr0")
            nc.vector.reciprocal_approx_fast(r0_t[:], sd_t[:])
            t1 = sb.tile([1, CHUNK], FP32, name=f"t1_{tagp}", tag="nt1")
            nc.vector.tensor_mul(t1[:], r0_t[:], r0_t[:])
            nc.vector.tensor_mul(t1[:], t1[:], v_t[:])
            nc.vector.tensor_scalar(t1[:], t1[:], -0.5, 1.5,
                                    op0=mybir.AluOpType.mult,
                                    op1=mybir.AluOpType.add)
            r_t = sb.tile([1, CHUNK], FP32, name=f"r_{tagp}", tag="nr")
            nc.vector.tensor_mul(r_t[:], r0_t[:], t1[:])
            return r_t

        # q tiles (head-packed: [rA32 iA32 rB32 iB32] per tile) cross phases
        q_f8 = [pAB.tile([128, CHUNK], FP8, name=f"qf8_{m}", tag="qf8",
                         bufs=NFT) for m in range(NFT)]
        # attn output, doublerow-paired: tile k holds feature blocks 2k, 2k+1
        attn_f8 = [p0.tile([128, 2 * CHUNK], FP8, name=f"attn_{k}",
                           tag="attn_f8", bufs=NK2) for k in range(NK2)]
        h_t = [p0.tile([128, CHUNK], FP32, name=f"h_{m}", tag="h", bufs=NFT)
               for m in range(NFT)]
        # normalized x, doublerow-paired fp8 (64*xn)
        xn_f8 = [pAB.tile([128, 2 * CHUNK], FP8, name=f"xn_{k}", tag="xn",
                          bufs=NK2) for k in range(NK2)]

        # chunked K/V gathers (fp8): a = heads 0-7 / hp 0-3, b = rest
        kg_a = dr.tile([GROUP * 512, CHUNK], FP8, name="kg_a")
        kg_b = dr.tile([GROUP * 512, CHUNK], FP8, name="kg_b")
        vg_a = dr.tile([GROUP * CHUNK, 512], FP8, name="vg_a")
        vg_b = dr.tile([GROUP * CHUNK, 512], FP8, name="vg_b")

        # ================= phase A: rmsnorm1 + K -> AGs, V -> AGs, Q =======
        with tc.tile_pool(name="pA", bufs=1) as pA, \
             tc.tile_pool(name="psA", bufs=1, space="PSUM") as psA:

            r_t = rmsnorm_stats(pA, psA, x_t, "n1")
            # broadcast 64*r over 128 partitions via PE (fp32 matmul, K=1)
            rb_ps = psA.tile([128, CHUNK], FP32, name="rb_n1", tag="nrb")
            nc.tensor.matmul(rb_ps[:], row64_t[:], r_t[:], start=True, stop=True)
            for i in range(NFT):
                o = xn_f8[i // 2][:, (i % 2) * CHUNK:(i % 2 + 1) * CHUNK]
                nc.vector.tensor_mul(o, x_t[i][:], rb_ps[:])
                if apply_attn_w:
                    nc.vector.tensor_scalar_mul(o, o, attnw_t[:, i:i + 1])
            xn_3d = [t.rearrange("p (j c) -> p j c", j=2) for t in xn_f8]

            def load_wmat(dram_ap, name, pool, tag="wmat", bufs=3 * NK2):
                tiles = []
                for k in range(NK2):
                    t = pool.tile([128, 2 * DIM], FP8, name=f"{name}_{k}",
                                  tag=tag, bufs=bufs)
                    nc.sync.dma_start(t[:], dram_ap[k * 128:(k + 1) * 128, :])
                    tiles.append(t)
                return [t.rearrange("p (j m) -> p j m", j=2) for t in tiles]

            def rope_tensor(w3d, outs, out_name, out_bufs, ggs, sb=None,
                            ps=None, ps_bufs=4, ps_tag="qkv_ps"):
                sb = sb or pA
                ps = ps or psA
                """Weight cols are [r-tiles 0..3 | i-tiles 0..3] (4 heads per
                tile, 32 rows each).  Produces head-packed fp8 tiles
                [rA32 iA32 rB32 iB32] for K=64 scores matmuls."""
                res = {}
                for gg in ggs:
                    r_ps = ps.tile([128, CHUNK], FP32, name=f"{out_name}rps_{gg}",
                                   tag=ps_tag, bufs=ps_bufs)
                    i_ps = ps.tile([128, CHUNK], FP32, name=f"{out_name}ips_{gg}",
                                   tag=ps_tag, bufs=ps_bufs)
                    for k in range(NK2):
                        nc.tensor.matmul(r_ps[:],
                                         w3d[k][:, :, gg * 128:(gg + 1) * 128],
                                         xn_3d[k][:],
                                         start=(k == 0), stop=(k == NK2 - 1),
                                         perf_mode=DR)
                    for k in range(NK2):
                        nc.tensor.matmul(i_ps[:],
                                         w3d[k][:, :, 512 + gg * 128:512 + (gg + 1) * 128],
                                         xn_3d[k][:],
                                         start=(k == 0), stop=(k == NK2 - 1),
                                         perf_mode=DR)
                    t1 = sb.tile([128, CHUNK], BF16, name=f"{out_name}t1_{gg}",
                                 tag="rope_t1", bufs=2)
                    t2 = sb.tile([128, CHUNK], BF16, name=f"{out_name}t2_{gg}",
                                 tag="rope_t2", bufs=2)
                    t3 = sb.tile([128, CHUNK], BF16, name=f"{out_name}t3_{gg}",
                                 tag="rope_t3", bufs=2)
                    t4 = sb.tile([128, CHUNK], BF16, name=f"{out_name}t4_{gg}",
                                 tag="rope_t4", bufs=2)
                    nc.vector.tensor_mul(t1[:], r_ps[:], cos_t[:])
                    nc.vector.tensor_mul(t2[:], i_ps[:], sin_t[:])
                    nc.vector.tensor_mul(t3[:], r_ps[:], sin_t[:])
                    nc.vector.tensor_mul(t4[:], i_ps[:], cos_t[:])
                    for u in range(2):      # head-pair within the group
                        hp = 2 * gg + u
                        if outs is not None:
                            o = outs[hp]
                        else:
                            o = sb.tile([128, CHUNK], FP8,
                                        name=f"{out_name}_{hp}",
                                        tag=out_name, bufs=out_bufs)
                        for w in range(2):      # head within the pair
                            s0 = (2 * u + w) * 32
                            d0 = w * 64
                            nc.vector.tensor_sub(o[d0:d0 + 32, :],
                                                 t1[s0:s0 + 32, :],
                                                 t2[s0:s0 + 32, :])
                            nc.vector.tensor_add(o[d0 + 32:d0 + 64, :],
                                                 t3[s0:s0 + 32, :],
                                                 t4[s0:s0 + 32, :])
                        res[hp] = o
                return res

            # K first half -> AG (earliest collective), then Q (phase B
            # needs it first), then V/K second halves pipelined behind
            wk_3d = load_wmat(wkt, "wk", pAB, tag="wmkv", bufs=2 * NK2)
            wq_3d = load_wmat(wqt, "wq", pA, tag="wmat", bufs=NK2)
            k_f8_a = rope_tensor(wk_3d, None, "kf8a", 4, ggs=(0, 1))
            bounce_ka = dr.tile([512, CHUNK], FP8, name="bounce_ka")
            for hp in (0, 1, 2, 3):
                nc.gpsimd.dma_start(bounce_ka[hp * 128:(hp + 1) * 128, :],
                                    k_f8_a[hp][:])
            nc.gpsimd.collective_compute(
                "AllGather", mybir.AluOpType.bypass, replica_groups=groups,
                ins=[bounce_ka.opt()], outs=[kg_a.opt()])

            wv_3d = load_wmat(wvt, "wv", pAB, tag="wmkv", bufs=2 * NK2)

            def v_half(n2, sb=None, ps=None, ps_bufs=4, ps_tag="qkv_ps"):
                sb = sb or pA
                ps = ps or psA
                bounce_v = dr.tile([CHUNK, 512], FP8, name=f"bounce_v{n2}")
                for t4_ in range(CHUNK // 128):
                    v_ps = ps.tile([128, 512], FP32, name=f"vps_{t4_}_{n2}",
                                   tag=ps_tag, bufs=ps_bufs)
                    for k in range(NK2):
                        nc.tensor.matmul(v_ps[:],
                                         xn_3d[k][:, :, t4_ * 128:(t4_ + 1) * 128],
                                         wv_3d[k][:, :, n2 * 512:(n2 + 1) * 512],
                                         start=(k == 0), stop=(k == NK2 - 1),
                                         perf_mode=DR)
                    v_f8 = sb.tile([128, 512], FP8, name=f"vf8_{t4_}_{n2}",
                                   tag="vf8", bufs=2)
                    nc.vector.tensor_copy(v_f8[:], v_ps[:])
                    nc.gpsimd.dma_start(
                        bounce_v[t4_ * 128:(t4_ + 1) * 128, :], v_f8[:])
                nc.gpsimd.collective_compute(
                    "AllGather", mybir.AluOpType.bypass, replica_groups=groups,
                    ins=[bounce_v.opt()],
                    outs=[(vg_a if n2 == 0 else vg_b).opt()])

            v_half(0)

            rope_tensor(wq_3d, q_f8, "qf8", NFT, ggs=(0, 1, 2, 3))
            rope_fn, vhalf_fn = rope_tensor, v_half

            if debug:
                for k in range(NK2):
                    nc.sync.dma_start(dbg["dbg_xn"][k * 128:(k + 1) * 128, :],
                                      xn_f8[k][:])
                for i in range(NFT):
                    nc.sync.dma_start(dbg["dbg_q"][i * 128:(i + 1) * 128, :],
                                      q_f8[i][:])

        # FFN w1/w3 tiles (first 12 f-blocks live in p0; DMAs are spread
        # through the phase-B loop so khh/vaug loads are never queued
        # behind them; the rest stream in phase C to fit SBUF)
        NW13_P0 = 12
        w13_tiles = []
        w13_dmas = []
        for f in range(NW13_P0):
            w1f = p0.tile([128, DIM], BF16, name=f"w1f_{f}", tag="w13",
                          bufs=2 * NW13_P0)
            w13_dmas.append((w1f, w1t[f * 128:(f + 1) * 128, :]))
            w3f = p0.tile([128, DIM], BF16, name=f"w3f_{f}", tag="w13",
                          bufs=2 * NW13_P0)
            w13_dmas.append((w3f, w3t[f * 128:(f + 1) * 128, :]))
            w13_tiles.append((w1f, w3f))

        if debug:
            nc.sync.dma_start(dbg["dbg_kga"][:], kg_a[:])
            nc.sync.dma_start(dbg["dbg_vga"][:], vg_a[:])
        kga_r = kg_a.rearrange("(r f) t -> r f t", f=512)
        kgb_r = kg_b.rearrange("(r f) t -> r f t", f=512)
        vga_r = vg_a.rearrange("(kt p) f -> p kt f", p=128)
        vgb_r = vg_b.rearrange("(kt p) f -> p kt f", p=128)

        # ================= phase B: attention (16 heads, 8 pairs) =========
        # PE queue discipline: scores(hp) | PV(hp-1) | scores(hp+1) | ... so
        # the in-order PE stream never waits on the softmax-normalize chain;
        # normalization happens in a batched tail on sums copied to SBUF.
        NROUND = 8   # rounds of 2 k-tiles each
        with tc.tile_pool(name="pB", bufs=1) as pB, \
             tc.tile_pool(name="psB", bufs=1, space="PSUM") as psB:
            khh_t, vaug_t, probs_t, un_t, s2_t = {}, {}, {}, {}, {}
            # full-array scratch matmuls slotted into the per-round ACT waits:
            # they keep the activity monitor's window free of idle gaps so the
            # PE clock stays at 8/8 instead of throttling to 4/8 (the K_b/V_b
            # PSUM buffers are idle after ~110us and absorb the writes)
            fill_n = [0]

            def pe_filler(n):
                for _ in range(n):
                    fps = psB.tile([128, 512], FP32,
                                   name=f"fill_{fill_n[0]}", tag="qkv2_ps",
                                   bufs=2)
                    fill_n[0] += 1
                    nc.tensor.matmul(fps[:], warm_bf[:, 0:128], warm_bf[:],
                                     start=True, stop=True,
                                     skip_group_check=True)


            def load_khh(hp):
                t = pB.tile([128, SEQ], FP8, name=f"khh_{hp}", tag="khh",
                            bufs=2)
                src = kga_r if hp < 4 else kgb_r
                f0 = (hp % 4) * 128
                for r in range(GROUP):
                    nc.sync.dma_start(t[:, r * CHUNK:(r + 1) * CHUNK],
                                      src[r, f0:f0 + 128, :])
                khh_t[hp] = t

            def load_vaug(h):
                # [128, kt, 80] padded so the doublerow pair stride (80B) is
                # 16B-aligned; col 64 holds the ones row for the prob sums
                vaug = pB.tile([128, NKT * 80], FP8, name=f"vaug_{h}",
                               tag="vaug", bufs=3)
                vr = vaug.rearrange("p (kt c) -> p kt c", c=80)
                src = vga_r if h < 8 else vgb_r
                c0 = (h % 8) * 64
                nc.gpsimd.dma_start(vr[:, :, 0:64], src[:, :, c0:c0 + 64])
                nc.vector.memset(vr[:, :, 64:65], 1.0)
                vaug_t[h] = vr

            def emit_scores(hp):
                khh = khh_t[hp]
                probss = []
                for w in range(2):
                    h = hp * 2 + w
                    probss.append(pB.tile([128, NKT * 512], FP8,
                                          name=f"probs_{h}", tag="probs",
                                          bufs=4))
                probs_t[hp] = probss
                if use_mask:
                    mrt = maskt.rearrange("(kt p) t -> p kt t", p=128)
                for rnd in range(NROUND):
                    scs = [psB.tile([128, 1024], FP32,
                                    name=f"sc_{hp}_{w}_{rnd}",
                                    tag="sc_ps", bufs=2) for w in range(2)]
                    for w in range(2):
                        b0 = w * 64
                        for j in range(2):
                            kt = rnd * 2 + j
                            nc.tensor.matmul(
                                scs[w][:, j * 512:(j + 1) * 512],
                                khh[b0:b0 + 64, kt * 128:(kt + 1) * 128],
                                q_f8[hp][b0:b0 + 64, :],
                                start=True, stop=True)
                    pe_filler(3)
                    for w in range(2):
                        h = hp * 2 + w
                        if use_mask:
                            mt = pB.tile([128, 1024], FP32,
                                         name=f"mt_{h}_{rnd}", tag="mt", bufs=2)
                            mt_r = mt.rearrange("p (j t) -> p j t", j=2)
                            for j in range(2):
                                nc.sync.dma_start(mt_r[:, j, :],
                                                  mrt[:, rnd * 2 + j, :])
                            nc.vector.tensor_scalar_mul(scs[w][:], scs[w][:],
                                                        ESCALE * 8.0)
                            nc.vector.tensor_add(scs[w][:], scs[w][:], mt[:])
                            nc.scalar.activation(
                                probs_t[hp][w][:, rnd * 1024:(rnd + 1) * 1024],
                                scs[w][:], AF.Exp, scale=0.125)
                        else:
                            nc.scalar.activation(
                                probs_t[hp][w][:, rnd * 1024:(rnd + 1) * 1024],
                                scs[w][:], AF.Exp, scale=ESCALE)
                if debug and hp == 0:
                    nc.sync.dma_start(dbg["dbg_p0"][:], probss[0][:])

            def emit_pv(hp):
                probss = probs_t.pop(hp)
                vaugs = [vaug_t.pop(hp * 2), vaug_t.pop(hp * 2 + 1)]
                s2 = pB.tile([1, 2 * CHUNK], FP32, name=f"s2_{hp}",
                             tag="s2", bufs=2)
                uns = []
                for w in range(2):
                    pr = probss[w].rearrange("p (kt t) -> p kt t", t=512)
                    pv_ps = psB.tile([65, CHUNK], FP32, name=f"pv_{hp}_{w}",
                                     tag="pv_ps", bufs=2)
                    for k2 in range(NKT // 2):
                        nc.tensor.matmul(pv_ps[:],
                                         vaugs[w][:, 2 * k2:2 * k2 + 2, 0:65],
                                         pr[:, 2 * k2:2 * k2 + 2, :],
                                         start=(k2 == 0),
                                         stop=(k2 == NKT // 2 - 1),
                                         perf_mode=DR)
                    un = pB.tile([64, CHUNK], BF16, name=f"un_{hp}_{w}",
                                 tag="un", bufs=4)
                    nc.vector.tensor_copy(un[:], pv_ps[0:64, :])
                    nc.vector.tensor_copy(s2[0:1, w * CHUNK:(w + 1) * CHUNK],
                                          pv_ps[64:65, :])
                    uns.append(un)
                un_t[hp] = uns
                s2_t[hp] = s2

            def emit_norm(hp):
                s2 = s2_t.pop(hp)
                r2p = pB.tile([1, 2 * CHUNK], FP32, name=f"r2p_{hp}",
                              tag="r2p", bufs=1)
                nc.vector.reciprocal_approx_fast(r2p[:], s2[:])
                r2v = pB.tile([2, CHUNK], FP32, name=f"r2v_{hp}",
                              tag="r2v", bufs=2)
                nc.sync.dma_start(
                    r2v[:], r2p.rearrange("o (j t) -> o j t", j=2))
                rb_ps = psB.tile([128, CHUNK], FP32, name=f"rbp_{hp}",
                                 tag="sc_ps", bufs=2)
                nc.tensor.matmul(rb_ps[:], e2_t[:], r2v[:],
                                 start=True, stop=True)
                af = attn_f8[hp // 2][:, (hp % 2) * CHUNK:(hp % 2 + 1) * CHUNK]
                u0, u1 = un_t.pop(hp)
                nc.vector.tensor_mul(af[0:64, :], u0[:], rb_ps[0:64, :])
                nc.vector.tensor_mul(af[64:128, :], u1[:], rb_ps[64:128, :])

            load_khh(0)
            load_khh(1)
            # --- second K/V halves computed here: their rope/copies reuse
            # phase-A1 SBUF (barrier on A1's early DVE tail only) and their
            # AllGathers stream while the first score rounds run ---
            k_f8_b = rope_fn(wk_3d, None, "kf8b", 4, ggs=(2, 3), sb=pB,
                             ps=psB, ps_bufs=2, ps_tag="qkv2_ps")
            bounce_kb = dr.tile([512, CHUNK], FP8, name="bounce_kb")
            for hp in (4, 5, 6, 7):
                nc.gpsimd.dma_start(bounce_kb[(hp - 4) * 128:(hp - 3) * 128, :],
                                    k_f8_b[hp][:])
            nc.gpsimd.collective_compute(
                "AllGather", mybir.AluOpType.bypass, replica_groups=groups,
                ins=[bounce_kb.opt()], outs=[kg_b.opt()])
            vhalf_fn(1, sb=pB, ps=psB, ps_bufs=2, ps_tag="qkv2_ps")
            for h in range(6):
                load_vaug(h)
            NP = NHEAD // 2
            for hp in range(NP):
                if hp + 2 < NP:
                    load_khh(hp + 2)
                for h in (hp * 2 + 6, hp * 2 + 7):
                    if h < NHEAD:
                        load_vaug(h)
                for t_, src_ in w13_dmas[hp * 3:(hp + 1) * 3]:
                    nc.sync.dma_start(t_[:], src_)
                emit_scores(hp)
                for t_, src_ in w13_dmas[24 + hp * 3:24 + (hp + 1) * 3]:
                    nc.sync.dma_start(t_[:], src_)
                if hp >= 1:
                    emit_pv(hp - 1)
                if hp >= 2:
                    emit_norm(hp - 2)
                if hp >= 2:
                    khh_t.pop(hp - 2, None)
            emit_norm(NP - 2)
            emit_pv(NP - 1)
            emit_norm(NP - 1)

        ctxAB.close()   # release phase-A/B-only SBUF before the FFN

        # ================= phase C: wo + residual + rmsnorm2 ==============
        attn_3d = [t.rearrange("p (j c) -> p j c", j=2) for t in attn_f8]
        wo_3d = [t.rearrange("p (j m) -> p j m", j=2) for t in wo_sb]
        hn = []
        with tc.tile_pool(name="pCD", bufs=1) as pCD:
          with tc.tile_pool(name="psC", bufs=1, space="PSUM") as psC:
            pC = pCD
            for f in range(NW13_P0, NMID):
                w1f = pCD.tile([128, DIM], BF16, name=f"w1f_{f}", tag="w13b",
                               bufs=2 * (NMID - NW13_P0))
                nc.sync.dma_start(w1f[:], w1t[f * 128:(f + 1) * 128, :])
                w3f = pCD.tile([128, DIM], BF16, name=f"w3f_{f}", tag="w13b",
                               bufs=2 * (NMID - NW13_P0))
                nc.sync.dma_start(w3f[:], w3t[f * 128:(f + 1) * 128, :])
                w13_tiles.append((w1f, w3f))
            for m in range(NFT):
                wo_ps = psC.tile([128, CHUNK], FP32, name=f"wops_{m}",
                                 tag="wo_ps", bufs=3)
                for k in range(NK2):
                    nc.tensor.matmul(wo_ps[:],
                                     wo_3d[k][:, :, m * 128:(m + 1) * 128],
                                     attn_3d[k][:],
                                     start=(k == 0), stop=(k == NK2 - 1),
                                     perf_mode=DR)
                tmp = pC.tile([128, CHUNK], BF16, name=f"wotmp_{m}",
                              tag="wotmp", bufs=3)
                nc.scalar.activation(tmp[:], wo_ps[:], AF.Copy,
                                     scale=1.0 / XSCALE)
                nc.vector.tensor_add(h_t[m][:], x_t[m][:], tmp[:])

            r2_t = rmsnorm_stats(pC, psC, h_t, "n2")
            rb2_ps = psC.tile([128, CHUNK], FP32, name="rb_n2", tag="wo_ps",
                              bufs=3)
            nc.tensor.matmul(rb2_ps[:], onesr_t[:], r2_t[:],
                             start=True, stop=True)
            for i in range(NFT):
                o = p0.tile([128, CHUNK], BF16, name=f"hn_{i}", tag="hn",
                            bufs=NFT)
                nc.vector.tensor_mul(o[:], h_t[i][:], rb2_ps[:])
                if apply_ffn_w:
                    nc.vector.tensor_scalar_mul(o[:], o[:], ffnw_t[:, i:i + 1])
                hn.append(o)
            if debug:
                for k in range(NK2):
                    nc.sync.dma_start(dbg["dbg_attn"][k * 128:(k + 1) * 128, :],
                                      attn_f8[k][:])
                for i in range(NFT):
                    nc.sync.dma_start(dbg["dbg_h"][i * 128:(i + 1) * 128, :], h_t[i][:])
                    nc.sync.dma_start(dbg["dbg_hn"][i * 128:(i + 1) * 128, :], hn[i][:])

          # =============== phase D: SwiGLU FFN (bf16) =====================
          with tc.tile_pool(name="psD", bufs=1, space="PSUM") as psD:
            pD = pCD
            mid = []
            for f in range(NMID):
                w1f, w3f = w13_tiles[f]
                g_ps = psD.tile([128, CHUNK], FP32, name=f"gps_{f}",
                                tag="g_ps", bufs=2)
                for k in range(NFT):
                    nc.tensor.matmul(g_ps[:], w1f[:, k * 128:(k + 1) * 128],
                                     hn[k][:],
                                     start=(k == 0), stop=(k == NFT - 1))
                sg = pD.tile([128, CHUNK], BF16, name=f"sg_{f}", tag="sg",
                             bufs=2)
                nc.scalar.activation(sg[:], g_ps[:], AF.Silu)
                u_ps = psD.tile([128, CHUNK], FP32, name=f"ups_{f}",
                                tag="u_ps", bufs=2)
                for k in range(NFT):
                    nc.tensor.matmul(u_ps[:], w3f[:, k * 128:(k + 1) * 128],
                                     hn[k][:],
                                     start=(k == 0), stop=(k == NFT - 1))
                md = pD.tile([128, CHUNK], BF16, name=f"mid_{f}", tag="mid",
                             bufs=NMID)
                nc.vector.tensor_mul(md[:], sg[:], u_ps[:])
                mid.append(md)

            for m in range(NFT):
                w2m = pD.tile([128, NMID * 128], BF16, name=f"w2m_{m}",
                              tag="w2m", bufs=2)
                nc.sync.dma_start(w2m[:], w2t[m * 128:(m + 1) * 128, :])
                o_ps = psD.tile([128, CHUNK], FP32, name=f"ops_{m}",
                                tag="o_ps", bufs=2)
                for f in range(NMID):
                    nc.tensor.matmul(o_ps[:], w2m[:, f * 128:(f + 1) * 128],
                                     mid[f][:],
                                     start=(f == 0), stop=(f == NMID - 1))
                ot = pD.tile([128, CHUNK], FP32, name=f"ot_{m}", tag="ot",
                             bufs=2)
                nc.vector.tensor_add(ot[:], h_t[m][:], o_ps[:])
                nc.sync.dma_start(out_fm[m * 128:(m + 1) * 128, :], ot[:])

    nc.compile()
    return nc


def _get_program(use_mask, apply_attn_w, apply_ffn_w, debug=False):
    key = (use_mask, apply_attn_w, apply_ffn_w, debug)
    if key not in _prog_cache:
        _prog_cache[key] = _build_program(*key)
    return _prog_cache[key]


def _rope_perm():
    """Row permutation: real (even) features of all heads first (4 tiles of
    4 heads x 32), then imag (odd) features in the same head order."""
    r_idx = np.concatenate([h * HD + 2 * np.arange(32) for h in range(NHEAD)])
    i_idx = np.concatenate([h * HD + 1 + 2 * np.arange(32) for h in range(NHEAD)])
    return np.concatenate([r_idx, i_idx])


def _tile_dr(w):
    """w (out DIM, in DIM) -> doublerow-tiled fp8 (4*128, 2*1024):
    block k rows = SBUF tile [p, (j m)] with value w[m, 256k + 128j + p]."""
    a = np.asarray(w, np.float32).T.reshape(NK2, 2, 128, DIM)  # [k, j, p, m]
    return np.ascontiguousarray(
        a.transpose(0, 2, 1, 3).reshape(NK2 * 128, 2 * DIM)).astype(FP8_NP)


def _tile_w13(w):
    """w (FFN, DIM) -> pre-tiled bf16 (FFN, DIM): block f rows = SBUF tile
    [p, (k c)] with value w.T[k*128+p, f*128+c]."""
    a = np.asarray(w, np.float32).reshape(NMID, 128, NFT, 128)  # [f, c, k, p]
    return np.ascontiguousarray(
        a.transpose(0, 3, 2, 1).reshape(NMID * 128, NFT * 128)).astype(BF16_NP)


def _tile_w2(w):
    """w (DIM, FFN) -> pre-tiled bf16 (DIM, FFN): block m rows = SBUF tile
    [p, (k c)] with value w.T[k*128+p, m*128+c]."""
    a = np.asarray(w, np.float32).reshape(NFT, 128, NMID, 128)  # [m, c, k, p]
    return np.ascontiguousarray(
        a.transpose(0, 3, 2, 1).reshape(NFT * 128, NMID * 128)).astype(BF16_NP)


def _prepare(inputs):
    hidden = np.ascontiguousarray(np.asarray(inputs["hidden_states_in"], np.float32))
    cos = np.asarray(inputs["freqs_cos"], np.float32)
    sin = np.asarray(inputs["freqs_sin"], np.float32)
    mask = np.asarray(inputs["mask"], np.float32)
    attn_w = np.asarray(inputs["attn_norm_w"], np.float32)
    ffn_w = np.asarray(inputs["ffn_norm_w"], np.float32)
    start_pos = int(np.asarray(inputs["start_pos"]))
    assert start_pos == 0, f"kernel only supports start_pos=0, got {start_pos}"

    use_mask = bool(np.any(mask))
    apply_attn_w = not bool(np.all(attn_w == 1.0))
    apply_ffn_w = not bool(np.all(ffn_w == 1.0))

    perm = _rope_perm()
    wq = np.asarray(inputs["wq"], np.float32)[perm, :]
    wk = np.asarray(inputs["wk"], np.float32)[perm, :]
    shared = {
        "wqt": _tile_dr(wq),
        "wkt": _tile_dr(wk),
        "wvt": _tile_dr(inputs["wv"]),
        "wot": _tile_dr(inputs["wo"]),
        "w1t": _tile_w13(inputs["w1"]),
        "w3t": _tile_w13(inputs["w3"]),
        "w2t": _tile_w2(inputs["w2"]),
        "ones_col": np.ones((128, 1), np.float32),
        "ones_row": np.ones((1, 128), np.float32),
        "row64": np.full((1, 128), XSCALE, np.float32),
    }
    e2 = np.zeros((2, 128), np.float32)
    e2[0, 0:64] = 1.0
    e2[1, 64:128] = 1.0
    shared["e2"] = e2
    if apply_attn_w:
        shared["attnw"] = attn_w.reshape(DIM, 1)
    if apply_ffn_w:
        shared["ffnw"] = ffn_w.reshape(DIM, 1)

    in_maps = []
    for c in range(NCORES):
        b = c // GROUP
        s0 = (c % GROUP) * CHUNK
        m = dict(shared)
        m["x_fm"] = np.ascontiguousarray(hidden[b, s0:s0 + CHUNK, :].T)
        cc = np.ascontiguousarray(cos[s0:s0 + CHUNK, :].T)  # (32, CHUNK)
        ss = np.ascontiguousarray(sin[s0:s0 + CHUNK, :].T)
        m["csa"] = np.ascontiguousarray(np.tile(cc, (4, 1)))  # cos, 4 heads/tile
        m["csb"] = np.ascontiguousarray(np.tile(ss, (4, 1)))  # sin
        if use_mask:
            m["maskt"] = np.ascontiguousarray(mask[b, s0:s0 + CHUNK, :].T)
        in_maps.append(m)
    return in_maps, (use_mask, apply_attn_w, apply_ffn_w)


def _assemble(results):
    out = np.empty((BSZ, SEQ, DIM), np.float32)
    for c in range(NCORES):
        b = c // GROUP
        s0 = (c % GROUP) * CHUNK
        out[b, s0:s0 + CHUNK, :] = results[c]["out_fm"].T
    return out


def run(inputs, trace=False, debug=False):
    in_maps, key = _prepare(inputs)
    nc = _get_program(*key, debug=debug)
    res = run_bass_kernel_spmd(nc, in_maps, core_ids=list(range(NCORES)),
                               trace=trace)
    return _assemble(res.results), res


def kernel(**inputs) -> np.ndarray:
    out, _ = run(inputs)
    return out


# revision 27
# speedup vs baseline: 1.1157x; 1.1157x over previous
"""Trainium2 Bass kernel for nn_AttentionBlock (dense transformer block).

Sharding: 8 cores = (2 batches x 4 seq-chunks of 512 tokens).
Each core: rmsnorm + QKV + rope for its 512-token chunk, chunked AllGather of
K/V (fp8) across its batch's 4 cores, full attention for its queries over all
16 heads, then wo + residual + rmsnorm + SwiGLU FFN for its rows.

dtypes: fp8e4m3 for QKV/wo weights+activations (DoubleRow matmuls), fp8 for
K/V gather, scores, probs and PV; bf16 for the FFN.  Scales: xn is written as
64*xn (folded into the rmsnorm broadcast), weights quantized at scale 1.0,
probs = 16*exp(score) via the exp bias; all scales cancel in the softmax
normalization except a single 1/64 on the wo PSUM evacuation.
"""

from contextlib import ExitStack

import numpy as np
import ml_dtypes

import concourse.bacc as bacc
import concourse.mybir as mybir
import concourse.tile as tile
from concourse.bass_utils import run_bass_kernel_spmd

DIM = 1024
NHEAD = 16
HD = 64
SEQ = 2048
BSZ = 2
FFN = 2816
EPS = 1e-6
NCORES = 8
GROUP = 4             # cores per batch (sequence-sharding group)
CHUNK = SEQ // GROUP  # 512 local tokens per core
NKT = SEQ // 128      # 16 key tiles
NFT = DIM // 128      # 8 feature tiles
NMID = FFN // 128     # 22 ffn hidden tiles
NK2 = DIM // 256      # 4 doublerow contraction steps over DIM

# NOTE: hw float8e4 is IEEE e4m3 (max finite 240, has inf) - keep scaled
# values comfortably below 240.
XSCALE = 32.0         # fp8 scale on normalized activations
ESCALE = 0.125 / (XSCALE * XSCALE)  # score descale folded into exp

FP32 = mybir.dt.float32
F32R = mybir.dt.float32r
BF16 = mybir.dt.bfloat16
FP8 = mybir.dt.float8e4
AF = mybir.ActivationFunctionType
DR = mybir.MatmulPerfMode.DoubleRow
BF16_NP = ml_dtypes.bfloat16
FP8_NP = ml_dtypes.float8_e4m3

_prog_cache = {}


def _build_program(use_mask: bool, apply_attn_w: bool, apply_ffn_w: bool,
                   debug: bool = False):
    nc = bacc.Bacc("TRN2", target_bir_lowering=False, debug=False,
                   num_devices=NCORES)

    # ---- DRAM I/O ----
    x_fm = nc.dram_tensor("x_fm", (DIM, CHUNK), FP32, kind="ExternalInput").ap()
    csa = nc.dram_tensor("csa", (128, CHUNK), FP32, kind="ExternalInput").ap()
    csb = nc.dram_tensor("csb", (128, CHUNK), FP32, kind="ExternalInput").ap()
    ones_col = nc.dram_tensor("ones_col", (128, 1), F32R, kind="ExternalInput").ap()
    ones_row = nc.dram_tensor("ones_row", (1, 128), FP32, kind="ExternalInput").ap()
    row64 = nc.dram_tensor("row64", (1, 128), FP32, kind="ExternalInput").ap()
    e2 = nc.dram_tensor("e2", (2, 128), FP32, kind="ExternalInput").ap()
    # qkv/wo weights: doublerow-tiled [4*128, 2*1024] fp8 (host-tiled)
    wqt = nc.dram_tensor("wqt", (NK2 * 128, 2 * DIM), FP8, kind="ExternalInput").ap()
    wkt = nc.dram_tensor("wkt", (NK2 * 128, 2 * DIM), FP8, kind="ExternalInput").ap()
    wvt = nc.dram_tensor("wvt", (NK2 * 128, 2 * DIM), FP8, kind="ExternalInput").ap()
    wot = nc.dram_tensor("wot", (NK2 * 128, 2 * DIM), FP8, kind="ExternalInput").ap()
    # w1t/w3t: pre-tiled [f, p, (k c)]; w2t: pre-tiled [m, p, (k c)] (host)
    w1t = nc.dram_tensor("w1t", (FFN, DIM), BF16, kind="ExternalInput").ap()
    w3t = nc.dram_tensor("w3t", (FFN, DIM), BF16, kind="ExternalInput").ap()
    w2t = nc.dram_tensor("w2t", (DIM, FFN), BF16, kind="ExternalInput").ap()
    if apply_attn_w:
        attnw = nc.dram_tensor("attnw", (DIM, 1), FP32, kind="ExternalInput").ap()
    if apply_ffn_w:
        ffnw = nc.dram_tensor("ffnw", (DIM, 1), FP32, kind="ExternalInput").ap()
    if use_mask:
        maskt = nc.dram_tensor("maskt", (SEQ, CHUNK), FP32, kind="ExternalInput").ap()
    out_fm = nc.dram_tensor("out_fm", (DIM, CHUNK), FP32, kind="ExternalOutput").ap()
    dbg = {}
    if debug:
        for nm, shape, ddt in [
                ("dbg_xn", (4 * 128, 2 * CHUNK), FP8),
                ("dbg_q", (DIM, CHUNK), FP8),
                ("dbg_kga", (GROUP * 512, CHUNK), FP8),
                ("dbg_vga", (GROUP * CHUNK, 512), FP8),
                ("dbg_p0", (128, NKT * 512), FP8),
                ("dbg_attn", (4 * 128, 2 * CHUNK), FP8),
                ("dbg_h", (DIM, CHUNK), FP32),
                ("dbg_hn", (DIM, CHUNK), BF16)]:
            dbg[nm] = nc.dram_tensor(nm, shape, ddt, kind="ExternalOutput").ap()

    groups = [list(range(GROUP)), list(range(GROUP, 2 * GROUP))]

    with tile.TileContext(nc) as tc, ExitStack() as ctx:
        # long-lived pool: constants + tiles that cross phase boundaries
        p0 = ctx.enter_context(tc.tile_pool(name="p0", bufs=1))
        dr = ctx.enter_context(tc.tile_pool(name="dr", bufs=1, space="DRAM"))
        # pool for tiles only needed through phase B (freed before the FFN)
        ctxAB = ExitStack()
        pAB = ctxAB.enter_context(tc.tile_pool(name="pAB", bufs=1))

        # fire the CC-stream warmup collective first: its ~30us firmware
        # startup runs while the DMAs and phase A compute proceed
        warm_in = dr.tile([128, 4], BF16, name="warm_in")
        warm_out = dr.tile([GROUP * 128, 4], BF16, name="warm_out")
        warm_sb = pAB.tile([128, 4], BF16, name="warm_sb")
        nc.vector.memset(warm_sb[:], 0.0)
        nc.sync.dma_start(warm_in[:], warm_sb[:])
        nc.gpsimd.collective_compute(
            "AllGather", mybir.AluOpType.bypass, replica_groups=groups,
            ins=[warm_in.opt()], outs=[warm_out.opt()])

        cos_t = pAB.tile([128, CHUNK], FP32, name="cos_t")
        sin_t = pAB.tile([128, CHUNK], FP32, name="sin_t")
        onesc_t = p0.tile([128, 1], F32R, name="onesc_t")
        onesr_t = p0.tile([1, 128], FP32, name="onesr_t")
        row64_t = p0.tile([1, 128], FP32, name="row64_t")
        e2_t = p0.tile([2, 128], FP32, name="e2_t")
        eps_t = p0.tile([1, 1], FP32, name="eps_t")
        nc.vector.memset(eps_t[:], EPS)
        nc.sync.dma_start(cos_t[:], csa[:])
        nc.sync.dma_start(sin_t[:], csb[:])
        nc.sync.dma_start(onesc_t[:], ones_col[:])
        nc.sync.dma_start(onesr_t[:], ones_row[:])
        nc.sync.dma_start(row64_t[:], row64[:])
        nc.sync.dma_start(e2_t[:], e2[:])
        attnw_t = ffnw_t = None
        if apply_attn_w:
            attnw_t = p0.tile([128, NFT], FP32, name="attnw_t")
            nc.sync.dma_start(
                attnw_t[:], attnw.rearrange("(k p) o -> p (k o)", p=128))
        if apply_ffn_w:
            ffnw_t = p0.tile([128, NFT], FP32, name="ffnw_t")
            nc.sync.dma_start(
                ffnw_t[:], ffnw.rearrange("(k p) o -> p (k o)", p=128))

        # PE warm-up: a short dense bf16 matmul burst so HAM reaches 8/8
        # before the real pipeline starts
        warm_bf = pAB.tile([128, CHUNK], BF16, name="warm_bf")
        nc.vector.tensor_copy(warm_bf[:], cos_t[:])
        with tc.tile_pool(name="psW", bufs=1, space="PSUM") as psW:
            wu_ps = psW.tile([128, 512], FP32, name="wu_ps", tag="wu")
            for it in range(14):
                nc.tensor.matmul(wu_ps[:], warm_bf[:, 0:128], warm_bf[:],
                                 start=(it == 0), stop=(it == 13))

        # hidden chunk (feature-major, kept for the attention residual)
        x_t = []
        for i in range(NFT):
            t = p0.tile([128, CHUNK], FP32, name=f"x_{i}", tag="x", bufs=NFT)
            nc.sync.dma_start(t[:], x_fm[i * 128:(i + 1) * 128, :])
            x_t.append(t)

        # wo weights (doublerow fp8): load early, consumed in phase C
        wo_sb = []
        for k in range(NK2):
            t = p0.tile([128, 2 * DIM], FP8, name=f"wo_{k}", tag="wo", bufs=NK2)
            nc.sync.dma_start(t[:], wot[k * 128:(k + 1) * 128, :])
            wo_sb.append(t)

        def rmsnorm_stats(sb, ps, src_tiles, tagp):
            """src (8 fm fp32 tiles) -> [1, CHUNK] reciprocal rms (fp32)."""
            ss_ps = ps.tile([1, CHUNK], FP32, name=f"ss_{tagp}", tag="ss")
            for i in range(NFT):
                sq = sb.tile([128, CHUNK], F32R, name=f"sq_{tagp}_{i}",
                             tag="sq", bufs=4)
                if i % 2 == 0:
                    nc.scalar.activation(sq[:], src_tiles[i][:], AF.Square)
                else:
                    nc.vector.tensor_mul(sq[:], src_tiles[i][:],
                                         src_tiles[i][:])
                nc.tensor.matmul(ss_ps[:], onesc_t[:], sq[:],
                                 start=(i == 0), stop=(i == NFT - 1))
            # v = mean + eps ; r = rsqrt(v) with one Newton step
            v_t = sb.tile([1, CHUNK], FP32, name=f"v_{tagp}", tag="nv")
            nc.scalar.activation(v_t[:], ss_ps[:], AF.Copy,
                                 scale=1.0 / DIM, bias=EPS)
            sd_t = sb.tile([1, CHUNK], FP32, name=f"sd_{tagp}", tag="nsd")
            nc.scalar.activation(sd_t[:], ss_ps[:], AF.Sqrt,
                                 scale=1.0 / DIM, bias=eps_t[:])
            r0_t = sb.tile([1, CHUNK], FP32, name=f"r0_{tagp}", tag="nr0")
            nc.vector.reciprocal_approx_fast(r0_t[:], sd_t[:])
            t1 = sb.tile([1, CHUNK], FP32, name=f"t1_{tagp}", tag="nt1")
            nc.vector.tensor_mul(t1[:], r0_t[:], r0_t[:])
            nc.vector.tensor_mul(t1[:], t1[:], v_t[:])
            nc.vector.tensor_scalar(t1[:], t1[:], -0.5, 1.5,
                                    op0=mybir.AluOpType.mult,
                                    op1=mybir.AluOpType.add)
            r_t = sb.tile([1, CHUNK], FP32, name=f"r_{tagp}", tag="nr")
            nc.vector.tensor_mul(r_t[:], r0_t[:], t1[:])
            return r_t

        # q tiles (head-packed: [rA32 iA32 rB32 iB32] per tile) cross phases
        q_f8 = [pAB.tile([128, CHUNK], FP8, name=f"qf8_{m}", tag="qf8",
                         bufs=NFT) for m in range(NFT)]
        # attn output, doublerow-paired: tile k holds feature blocks 2k, 2k+1
        attn_f8 = [p0.tile([128, 2 * CHUNK], FP8, name=f"attn_{k}",
                           tag="attn_f8", bufs=NK2) for k in range(NK2)]
        h_t = [p0.tile([128, CHUNK], FP32, name=f"h_{m}", tag="h", bufs=NFT)
               for m in range(NFT)]
        # normalized x, doublerow-paired fp8 (64*xn)
        xn_f8 = [pAB.tile([128, 2 * CHUNK], FP8, name=f"xn_{k}", tag="xn",
                          bufs=NK2) for k in range(NK2)]

        # chunked K/V gathers (fp8): a = heads 0-7 / hp 0-3, b = rest
        kg_a = dr.tile([GROUP * 512, CHUNK], FP8, name="kg_a")
        kg_b = dr.tile([GROUP * 512, CHUNK], FP8, name="kg_b")
        vg_a = dr.tile([GROUP * CHUNK, 512], FP8, name="vg_a")
        vg_b = dr.tile([GROUP * CHUNK, 512], FP8, name="vg_b")

        # ================= phase A: rmsnorm1 + K -> AGs, V -> AGs, Q =======
        with tc.tile_pool(name="pA", bufs=1) as pA, \
             tc.tile_pool(name="psA", bufs=1, space="PSUM") as psA:

            r_t = rmsnorm_stats(pA, psA, x_t, "n1")
            # broadcast 64*r over 128 partitions via PE (fp32 matmul, K=1)
            rb_ps = psA.tile([128, CHUNK], FP32, name="rb_n1", tag="nrb")
            nc.tensor.matmul(rb_ps[:], row64_t[:], r_t[:], start=True, stop=True)
            for i in range(NFT):
                o = xn_f8[i // 2][:, (i % 2) * CHUNK:(i % 2 + 1) * CHUNK]
                nc.vector.tensor_mul(o, x_t[i][:], rb_ps[:])
                if apply_attn_w:
                    nc.vector.tensor_scalar_mul(o, o, attnw_t[:, i:i + 1])
            xn_3d = [t.rearrange("p (j c) -> p j c", j=2) for t in xn_f8]

            def load_wmat(dram_ap, name, pool, tag="wmat", bufs=3 * NK2):
                tiles = []
                for k in range(NK2):
                    t = pool.tile([128, 2 * DIM], FP8, name=f"{name}_{k}",
                                  tag=tag, bufs=bufs)
                    nc.sync.dma_start(t[:], dram_ap[k * 128:(k + 1) * 128, :])
                    tiles.append(t)
                return [t.rearrange("p (j m) -> p j m", j=2) for t in tiles]

            def rope_tensor(w3d, outs, out_name, out_bufs, ggs, sb=None,
                            ps=None, ps_bufs=4, ps_tag="qkv_ps"):
                sb = sb or pA
                ps = ps or psA
                """Weight cols are [r-tiles 0..3 | i-tiles 0..3] (4 heads per
                tile, 32 rows each).  Produces head-packed fp8 tiles
                [rA32 iA32 rB32 iB32] for K=64 scores matmuls."""
                res = {}
                for gg in ggs:
                    r_ps = ps.tile([128, CHUNK], FP32, name=f"{out_name}rps_{gg}",
                                   tag=ps_tag, bufs=ps_bufs)
                    i_ps = ps.tile([128, CHUNK], FP32, name=f"{out_name}ips_{gg}",
                                   tag=ps_tag, bufs=ps_bufs)
                    for k in range(NK2):
                        nc.tensor.matmul(r_ps[:],
                                         w3d[k][:, :, gg * 128:(gg + 1) * 128],
                                         xn_3d[k][:],
                                         start=(k == 0), stop=(k == NK2 - 1),
                                         perf_mode=DR)
                    for k in range(NK2):
                        nc.tensor.matmul(i_ps[:],
                                         w3d[k][:, :, 512 + gg * 128:512 + (gg + 1) * 128],
                                         xn_3d[k][:],
                                         start=(k == 0), stop=(k == NK2 - 1),
                                         perf_mode=DR)
                    t1 = sb.tile([128, CHUNK], BF16, name=f"{out_name}t1_{gg}",
                                 tag="rope_t1", bufs=2)
                    t2 = sb.tile([128, CHUNK], BF16, name=f"{out_name}t2_{gg}",
                                 tag="rope_t2", bufs=2)
                    t3 = sb.tile([128, CHUNK], BF16, name=f"{out_name}t3_{gg}",
                                 tag="rope_t3", bufs=2)
                    t4 = sb.tile([128, CHUNK], BF16, name=f"{out_name}t4_{gg}",
                                 tag="rope_t4", bufs=2)
                    nc.vector.tensor_mul(t1[:], r_ps[:], cos_t[:])
                    nc.vector.tensor_mul(t2[:], i_ps[:], sin_t[:])
                    nc.vector.tensor_mul(t3[:], r_ps[:], sin_t[:])
                    nc.vector.tensor_mul(t4[:], i_ps[:], cos_t[:])
                    for u in range(2):      # head-pair within the group
                        hp = 2 * gg + u
                        if outs is not None:
                            o = outs[hp]
                        else:
                            o = sb.tile([128, CHUNK], FP8,
                                        name=f"{out_name}_{hp}",
                                        tag=out_name, bufs=out_bufs)
                        for w in range(2):      # head within the pair
                            s0 = (2 * u + w) * 32
                            d0 = w * 64
                            nc.vector.tensor_sub(o[d0:d0 + 32, :],
                                                 t1[s0:s0 + 32, :],
                                                 t2[s0:s0 + 32, :])
                            nc.vector.tensor_add(o[d0 + 32:d0 + 64, :],
                                                 t3[s0:s0 + 32, :],
                                                 t4[s0:s0 + 32, :])
                        res[hp] = o
                return res

            # K first half -> AG (earliest collective), then Q (phase B
            # needs it first), then V/K second halves pipelined behind
            wk_3d = load_wmat(wkt, "wk", pAB, tag="wmkv", bufs=2 * NK2)
            wq_3d = load_wmat(wqt, "wq", pA, tag="wmat", bufs=NK2)
            k_f8_a = rope_tensor(wk_3d, None, "kf8a", 4, ggs=(0, 1))
            bounce_ka = dr.tile([512, CHUNK], FP8, name="bounce_ka")
            for hp in (0, 1, 2, 3):
                nc.gpsimd.dma_start(bounce_ka[hp * 128:(hp + 1) * 128, :],
                                    k_f8_a[hp][:])
            nc.gpsimd.collective_compute(
                "AllGather", mybir.AluOpType.bypass, replica_groups=groups,
                ins=[bounce_ka.opt()], outs=[kg_a.opt()])

            wv_3d = load_wmat(wvt, "wv", pAB, tag="wmkv", bufs=2 * NK2)

            def v_half(n2, sb=None, ps=None, ps_bufs=4, ps_tag="qkv_ps"):
                sb = sb or pA
                ps = ps or psA
                bounce_v = dr.tile([CHUNK, 512], FP8, name=f"bounce_v{n2}")
                for t4_ in range(CHUNK // 128):
                    v_ps = ps.tile([128, 512], FP32, name=f"vps_{t4_}_{n2}",
                                   tag=ps_tag, bufs=ps_bufs)
                    for k in range(NK2):
                        nc.tensor.matmul(v_ps[:],
                                         xn_3d[k][:, :, t4_ * 128:(t4_ + 1) * 128],
                                         wv_3d[k][:, :, n2 * 512:(n2 + 1) * 512],
                                         start=(k == 0), stop=(k == NK2 - 1),
                                         perf_mode=DR)
                    v_f8 = sb.tile([128, 512], FP8, name=f"vf8_{t4_}_{n2}",
                                   tag="vf8", bufs=2)
                    nc.vector.tensor_copy(v_f8[:], v_ps[:])
                    nc.gpsimd.dma_start(
                        bounce_v[t4_ * 128:(t4_ + 1) * 128, :], v_f8[:])
                nc.gpsimd.collective_compute(
                    "AllGather", mybir.AluOpType.bypass, replica_groups=groups,
                    ins=[bounce_v.opt()],
                    outs=[(vg_a if n2 == 0 else vg_b).opt()])

            v_half(0)

            rope_tensor(wq_3d, q_f8, "qf8", NFT, ggs=(0, 1, 2, 3))
            rope_fn, vhalf_fn = rope_tensor, v_half

            if debug:
                for k in range(NK2):
                    nc.sync.dma_start(dbg["dbg_xn"][k * 128:(k + 1) * 128, :],
                                      xn_f8[k][:])
                for i in range(NFT):
                    nc.sync.dma_start(dbg["dbg_q"][i * 128:(i + 1) * 128, :],
                                      q_f8[i][:])

        # FFN w1/w3 tiles (first 12 f-blocks live in p0; DMAs are spread
        # through the phase-B loop so khh/vaug loads are never queued
        # behind them; the rest stream in phase C to fit SBUF)
        NW13_P0 = 12
        w13_tiles = []
        w13_dmas = []
        for f in range(NW13_P0):
            w1f = p0.tile([128, DIM], BF16, name=f"w1f_{f}", tag="w13",
                          bufs=2 * NW13_P0)
            w13_dmas.append((w1f, w1t[f * 128:(f + 1) * 128, :]))
            w3f = p0.tile([128, DIM], BF16, name=f"w3f_{f}", tag="w13",
                          bufs=2 * NW13_P0)
            w13_dmas.append((w3f, w3t[f * 128:(f + 1) * 128, :]))
            w13_tiles.append((w1f, w3f))

        if debug:
            nc.sync.dma_start(dbg["dbg_kga"][:], kg_a[:])
            nc.sync.dma_start(dbg["dbg_vga"][:], vg_a[:])
        kga_r = kg_a.rearrange("(r f) t -> r f t", f=512)
        kgb_r = kg_b.rearrange("(r f) t -> r f t", f=512)
        vga_r = vg_a.rearrange("(kt p) f -> p kt f", p=128)
        vgb_r = vg_b.rearrange("(kt p) f -> p kt f", p=128)

        # ================= phase B: attention (16 heads, 8 pairs) =========
        # PE queue discipline: scores(hp) | PV(hp-1) | scores(hp+1) | ... so
        # the in-order PE stream never waits on the softmax-normalize chain;
        # normalization happens in a batched tail on sums copied to SBUF.
        NROUND = 8   # rounds of 2 k-tiles each
        with tc.tile_pool(name="pB", bufs=1) as pB, \
             tc.tile_pool(name="psB", bufs=1, space="PSUM") as psB:
            khh_t, vaug_t, probs_t, un_t, s2_t = {}, {}, {}, {}, {}


            def load_khh(hp):
                t = pB.tile([128, SEQ], FP8, name=f"khh_{hp}", tag="khh",
                            bufs=2)
                src = kga_r if hp < 4 else kgb_r
                f0 = (hp % 4) * 128
                for r in range(GROUP):
                    nc.sync.dma_start(t[:, r * CHUNK:(r + 1) * CHUNK],
                                      src[r, f0:f0 + 128, :])
                khh_t[hp] = t

            def load_vaug(h):
                # [128, kt, 80] padded so the doublerow pair stride (80B) is
                # 16B-aligned; col 64 holds the ones row for the prob sums
                vaug = pB.tile([128, NKT * 80], FP8, name=f"vaug_{h}",
                               tag="vaug", bufs=3)
                vr = vaug.rearrange("p (kt c) -> p kt c", c=80)
                src = vga_r if h < 8 else vgb_r
                c0 = (h % 8) * 64
                nc.gpsimd.dma_start(vr[:, :, 0:64], src[:, :, c0:c0 + 64])
                nc.vector.memset(vr[:, :, 64:65], 1.0)
                vaug_t[h] = vr

            def emit_scores(hp):
                khh = khh_t[hp]
                probss = []
                for w in range(2):
                    h = hp * 2 + w
                    probss.append(pB.tile([128, NKT * 512], FP8,
                                          name=f"probs_{h}", tag="probs",
                                          bufs=4))
                probs_t[hp] = probss
                if use_mask:
                    mrt = maskt.rearrange("(kt p) t -> p kt t", p=128)
                for rnd in range(NROUND):
                    scs = [psB.tile([128, 1024], FP32,
                                    name=f"sc_{hp}_{w}_{rnd}",
                                    tag="sc_ps", bufs=2) for w in range(2)]
                    for w in range(2):
                        b0 = w * 64
                        for j in range(2):
                            kt = rnd * 2 + j
                            nc.tensor.matmul(
                                scs[w][:, j * 512:(j + 1) * 512],
                                khh[b0:b0 + 64, kt * 128:(kt + 1) * 128],
                                q_f8[hp][b0:b0 + 64, :],
                                start=True, stop=True)
                    for w in range(2):
                        h = hp * 2 + w
                        if use_mask:
                            mt = pB.tile([128, 1024], FP32,
                                         name=f"mt_{h}_{rnd}", tag="mt", bufs=2)
                            mt_r = mt.rearrange("p (j t) -> p j t", j=2)
                            for j in range(2):
                                nc.sync.dma_start(mt_r[:, j, :],
                                                  mrt[:, rnd * 2 + j, :])
                            nc.vector.tensor_scalar_mul(scs[w][:], scs[w][:],
                                                        ESCALE * 8.0)
                            nc.vector.tensor_add(scs[w][:], scs[w][:], mt[:])
                            nc.scalar.activation(
                                probs_t[hp][w][:, rnd * 1024:(rnd + 1) * 1024],
                                scs[w][:], AF.Exp, scale=0.125)
                        else:
                            nc.scalar.activation(
                                probs_t[hp][w][:, rnd * 1024:(rnd + 1) * 1024],
                                scs[w][:], AF.Exp, scale=ESCALE)
                if debug and hp == 0:
                    nc.sync.dma_start(dbg["dbg_p0"][:], probss[0][:])

            def emit_pv(hp):
                probss = probs_t.pop(hp)
                vaugs = [vaug_t.pop(hp * 2), vaug_t.pop(hp * 2 + 1)]
                s2 = pB.tile([1, 2 * CHUNK], FP32, name=f"s2_{hp}",
                             tag="s2", bufs=2)
                uns = []
                for w in range(2):
                    pr = probss[w].rearrange("p (kt t) -> p kt t", t=512)
                    pv_ps = psB.tile([65, CHUNK], FP32, name=f"pv_{hp}_{w}",
                                     tag="pv_ps", bufs=2)
                    for k2 in range(NKT // 2):
                        nc.tensor.matmul(pv_ps[:],
                                         vaugs[w][:, 2 * k2:2 * k2 + 2, 0:65],
                                         pr[:, 2 * k2:2 * k2 + 2, :],
                                         start=(k2 == 0),
                                         stop=(k2 == NKT // 2 - 1),
                                         perf_mode=DR)
                    un = pB.tile([64, CHUNK], BF16, name=f"un_{hp}_{w}",
                                 tag="un", bufs=4)
                    nc.vector.tensor_copy(un[:], pv_ps[0:64, :])
                    nc.vector.tensor_copy(s2[0:1, w * CHUNK:(w + 1) * CHUNK],
                                          pv_ps[64:65, :])
                    uns.append(un)
                un_t[hp] = uns
                s2_t[hp] = s2

            def emit_norm(hp):
                s2 = s2_t.pop(hp)
                r2p = pB.tile([1, 2 * CHUNK], FP32, name=f"r2p_{hp}",
                              tag="r2p", bufs=1)
                nc.vector.reciprocal_approx_fast(r2p[:], s2[:])
                r2v = pB.tile([2, CHUNK], FP32, name=f"r2v_{hp}",
                              tag="r2v", bufs=2)
                nc.sync.dma_start(
                    r2v[:], r2p.rearrange("o (j t) -> o j t", j=2))
                rb_ps = psB.tile([128, CHUNK], FP32, name=f"rbp_{hp}",
                                 tag="sc_ps", bufs=2)
                nc.tensor.matmul(rb_ps[:], e2_t[:], r2v[:],
                                 start=True, stop=True)
                af = attn_f8[hp // 2][:, (hp % 2) * CHUNK:(hp % 2 + 1) * CHUNK]
                u0, u1 = un_t.pop(hp)
                nc.vector.tensor_mul(af[0:64, :], u0[:], rb_ps[0:64, :])
                nc.vector.tensor_mul(af[64:128, :], u1[:], rb_ps[64:128, :])

            load_khh(0)
            load_khh(1)
            # --- second K/V halves computed here: their rope/copies reuse
            # phase-A1 SBUF (barrier on A1's early DVE tail only) and their
            # AllGathers stream while the first score rounds run ---
            k_f8_b = rope_fn(wk_3d, None, "kf8b", 4, ggs=(2, 3), sb=pB,
                             ps=psB, ps_bufs=2, ps_tag="qkv2_ps")
            bounce_kb = dr.tile([512, CHUNK], FP8, name="bounce_kb")
            for hp in (4, 5, 6, 7):
                nc.gpsimd.dma_start(bounce_kb[(hp - 4) * 128:(hp - 3) * 128, :],
                                    k_f8_b[hp][:])
            nc.gpsimd.collective_compute(
                "AllGather", mybir.AluOpType.bypass, replica_groups=groups,
                ins=[bounce_kb.opt()], outs=[kg_b.opt()])
            vhalf_fn(1, sb=pB, ps=psB, ps_bufs=2, ps_tag="qkv2_ps")
            for h in range(6):
                load_vaug(h)
            NP = NHEAD // 2
            for hp in range(NP):
                if hp + 2 < NP:
                    load_khh(hp + 2)
                for h in (hp * 2 + 6, hp * 2 + 7):
                    if h < NHEAD:
                        load_vaug(h)
                for t_, src_ in w13_dmas[hp * 3:(hp + 1) * 3]:
                    nc.sync.dma_start(t_[:], src_)
                emit_scores(hp)
                for t_, src_ in w13_dmas[24 + hp * 3:24 + (hp + 1) * 3]:
                    nc.sync.dma_start(t_[:], src_)
                if hp >= 1:
                    emit_pv(hp - 1)
                if hp >= 2:
                    emit_norm(hp - 2)
                if hp >= 2:
                    khh_t.pop(hp - 2, None)
            emit_norm(NP - 2)
            emit_pv(NP - 1)
            emit_norm(NP - 1)

        ctxAB.close()   # release phase-A/B-only SBUF before the FFN

        # ================= phase C: wo + residual + rmsnorm2 ==============
        attn_3d = [t.rearrange("p (j c) -> p j c", j=2) for t in attn_f8]
        wo_3d = [t.rearrange("p (j m) -> p j m", j=2) for t in wo_sb]
        hn = []
        with tc.tile_pool(name="pCD", bufs=1) as pCD:
          with tc.tile_pool(name="psC", bufs=1, space="PSUM") as psC:
            pC = pCD
            for f in range(NW13_P0, NMID):
                w1f = pCD.tile([128, DIM], BF16, name=f"w1f_{f}", tag="w13b",
                               bufs=2 * (NMID - NW13_P0))
                nc.sync.dma_start(w1f[:], w1t[f * 128:(f + 1) * 128, :])
                w3f = pCD.tile([128, DIM], BF16, name=f"w3f_{f}", tag="w13b",
                               bufs=2 * (NMID - NW13_P0))
                nc.sync.dma_start(w3f[:], w3t[f * 128:(f + 1) * 128, :])
                w13_tiles.append((w1f, w3f))
            for m in range(NFT):
                wo_ps = psC.tile([128, CHUNK], FP32, name=f"wops_{m}",
                                 tag="wo_ps", bufs=3)
                for k in range(NK2):
                    nc.tensor.matmul(wo_ps[:],
                                     wo_3d[k][:, :, m * 128:(m + 1) * 128],
                                     attn_3d[k][:],
                                     start=(k == 0), stop=(k == NK2 - 1),
                                     perf_mode=DR)
                tmp = pC.tile([128, CHUNK], BF16, name=f"wotmp_{m}",
                              tag="wotmp", bufs=3)
                nc.scalar.activation(tmp[:], wo_ps[:], AF.Copy,
                                     scale=1.0 / XSCALE)
                nc.vector.tensor_add(h_t[m][:], x_t[m][:], tmp[:])

            r2_t = rmsnorm_stats(pC, psC, h_t, "n2")
            rb2_ps = psC.tile([128, CHUNK], FP32, name="rb_n2", tag="wo_ps",
                              bufs=3)
            nc.tensor.matmul(rb2_ps[:], onesr_t[:], r2_t[:],
                             start=True, stop=True)
            for i in range(NFT):
                o = p0.tile([128, CHUNK], BF16, name=f"hn_{i}", tag="hn",
                            bufs=NFT)
                nc.vector.tensor_mul(o[:], h_t[i][:], rb2_ps[:])
                if apply_ffn_w:
                    nc.vector.tensor_scalar_mul(o[:], o[:], ffnw_t[:, i:i + 1])
                hn.append(o)
            if debug:
                for k in range(NK2):
                    nc.sync.dma_start(dbg["dbg_attn"][k * 128:(k + 1) * 128, :],
                                      attn_f8[k][:])
                for i in range(NFT):
                    nc.sync.dma_start(dbg["dbg_h"][i * 128:(i + 1) * 128, :], h_t[i][:])
                    nc.sync.dma_start(dbg["dbg_hn"][i * 128:(i + 1) * 128, :], hn[i][:])

          # =============== phase D: SwiGLU FFN (bf16) =====================
          with tc.tile_pool(name="psD", bufs=1, space="PSUM") as psD:
            pD = pCD
            mid = []
            for f in range(NMID):
                w1f, w3f = w13_tiles[f]
                g_ps = psD.tile([128, CHUNK], FP32, name=f"gps_{f}",
                                tag="g_ps", bufs=2)
                for k in range(NFT):
                    nc.tensor.matmul(g_ps[:], w1f[:, k * 128:(k + 1) * 128],
                                     hn[k][:],
                                     start=(k == 0), stop=(k == NFT - 1))
                sg = pD.tile([128, CHUNK], BF16, name=f"sg_{f}", tag="sg",
                             bufs=2)
                nc.scalar.activation(sg[:], g_ps[:], AF.Silu)
                u_ps = psD.tile([128, CHUNK], FP32, name=f"ups_{f}",
                                tag="u_ps", bufs=2)
                for k in range(NFT):
                    nc.tensor.matmul(u_ps[:], w3f[:, k * 128:(k + 1) * 128],
                                     hn[k][:],
                                     start=(k == 0), stop=(k == NFT - 1))
                md = pD.tile([128, CHUNK], BF16, name=f"mid_{f}", tag="mid",
                             bufs=NMID)
                nc.vector.tensor_mul(md[:], sg[:], u_ps[:])
                mid.append(md)

            for m in range(NFT):
                w2m = pD.tile([128, NMID * 128], BF16, name=f"w2m_{m}",
                              tag="w2m", bufs=2)
                nc.sync.dma_start(w2m[:], w2t[m * 128:(m + 1) * 128, :])
                o_ps = psD.tile([128, CHUNK], FP32, name=f"ops_{m}",
                                tag="o_ps", bufs=2)
                for f in range(NMID):
                    nc.tensor.matmul(o_ps[:], w2m[:, f * 128:(f + 1) * 128],
                                     mid[f][:],
                                     start=(f == 0), stop=(f == NMID - 1))
                ot = pD.tile([128, CHUNK], FP32, name=f"ot_{m}", tag="ot",
                             bufs=2)
                nc.vector.tensor_add(ot[:], h_t[m][:], o_ps[:])
                nc.sync.dma_start(out_fm[m * 128:(m + 1) * 128, :], ot[:])

    nc.compile()
    return nc


def _get_program(use_mask, apply_attn_w, apply_ffn_w, debug=False):
    key = (use_mask, apply_attn_w, apply_ffn_w, debug)
    if key not in _prog_cache:
        _prog_cache[key] = _build_program(*key)
    return _prog_cache[key]


def _rope_perm():
    """Row permutation: real (even) features of all heads first (4 tiles of
    4 heads x 32), then imag (odd) features in the same head order."""
    r_idx = np.concatenate([h * HD + 2 * np.arange(32) for h in range(NHEAD)])
    i_idx = np.concatenate([h * HD + 1 + 2 * np.arange(32) for h in range(NHEAD)])
    return np.concatenate([r_idx, i_idx])


def _tile_dr(w):
    """w (out DIM, in DIM) -> doublerow-tiled fp8 (4*128, 2*1024):
    block k rows = SBUF tile [p, (j m)] with value w[m, 256k + 128j + p]."""
    a = np.asarray(w, np.float32).T.reshape(NK2, 2, 128, DIM)  # [k, j, p, m]
    return np.ascontiguousarray(
        a.transpose(0, 2, 1, 3).reshape(NK2 * 128, 2 * DIM)).astype(FP8_NP)


def _tile_w13(w):
    """w (FFN, DIM) -> pre-tiled bf16 (FFN, DIM): block f rows = SBUF tile
    [p, (k c)] with value w.T[k*128+p, f*128+c]."""
    a = np.asarray(w, np.float32).reshape(NMID, 128, NFT, 128)  # [f, c, k, p]
    return np.ascontiguousarray(
        a.transpose(0, 3, 2, 1).reshape(NMID * 128, NFT * 128)).astype(BF16_NP)


def _tile_w2(w):
    """w (DIM, FFN) -> pre-tiled bf16 (DIM, FFN): block m rows = SBUF tile
    [p, (k c)] with value w.T[k*128+p, m*128+c]."""
    a = np.asarray(w, np.float32).reshape(NFT, 128, NMID, 128)  # [m, c, k, p]
    return np.ascontiguousarray(
        a.transpose(0, 3, 2, 1).reshape(NFT * 128, NMID * 128)).astype(BF16_NP)


def _prepare(inputs):
    hidden = np.ascontiguousarray(np.asarray(inputs["hidden_states_in"], np.float32))
    cos = np.asarray(inputs["freqs_cos"], np.float32)
    sin = np.asarray(inputs["freqs_sin"], np.float32)
    mask = np.asarray(inputs["mask"], np.float32)
    attn_w = np.asarray(inputs["attn_norm_w"], np.float32)
    ffn_w = np.asarray(inputs["ffn_norm_w"], np.float32)
    start_pos = int(np.asarray(inputs["start_pos"]))
    assert start_pos == 0, f"kernel only supports start_pos=0, got {start_pos}"

    use_mask = bool(np.any(mask))
    apply_attn_w = not bool(np.all(attn_w == 1.0))
    apply_ffn_w = not bool(np.all(ffn_w == 1.0))

    perm = _rope_perm()
    wq = np.asarray(inputs["wq"], np.float32)[perm, :]
    wk = np.asarray(inputs["wk"], np.float32)[perm, :]
    shared = {
        "wqt": _tile_dr(wq),
        "wkt": _tile_dr(wk),
        "wvt": _tile_dr(inputs["wv"]),
        "wot": _tile_dr(inputs["wo"]),
        "w1t": _tile_w13(inputs["w1"]),
        "w3t": _tile_w13(inputs["w3"]),
        "w2t": _tile_w2(inputs["w2"]),
        "ones_col": np.ones((128, 1), np.float32),
        "ones_row": np.ones((1, 128), np.float32),
        "row64": np.full((1, 128), XSCALE, np.float32),
    }
    e2 = np.zeros((2, 128), np.float32)
    e2[0, 0:64] = 1.0
    e2[1, 64:128] = 1.0
    shared["e2"] = e2
    if apply_attn_w:
        shared["attnw"] = attn_w.reshape(DIM, 1)
    if apply_ffn_w:
        shared["ffnw"] = ffn_w.reshape(DIM, 1)

    in_maps = []
    for c in range(NCORES):
        b = c // GROUP
        s0 = (c % GROUP) * CHUNK
        m = dict(shared)
        m["x_fm"] = np.ascontiguousarray(hidden[b, s0:s0 + CHUNK, :].T)
        cc = np.ascontiguousarray(cos[s0:s0 + CHUNK, :].T)  # (32, CHUNK)
        ss = np.ascontiguousarray(sin[s0:s0 + CHUNK, :].T)
        m["csa"] = np.ascontiguousarray(np.tile(cc, (4, 1)))  # cos, 4 heads/tile
        m["csb"] = np.ascontiguousarray(np.tile(ss, (4, 1)))  # sin
        if use_mask:
            m["maskt"] = np.ascontiguousarray(mask[b, s0:s0 + CHUNK, :].T)
        in_maps.append(m)
    return in_maps, (use_mask, apply_attn_w, apply_ffn_w)


def _assemble(results):
    out = np.empty((BSZ, SEQ, DIM), np.float32)
    for c in range(NCORES):
        b = c // GROUP
        s0 = (c % GROUP) * CHUNK
        out[b, s0:s0 + CHUNK, :] = results[c]["out_fm"].T
    return out


def run(inputs, trace=False, debug=False):
    in_maps, key = _prepare(inputs)
    nc = _get_program(*key, debug=debug)
    res = run_bass_kernel_spmd(nc, in_maps, core_ids=list(range(NCORES)),
                               trace=trace)
    return _assemble(res.results), res


def kernel(**inputs) -> np.ndarray:
    out, _ = run(inputs)
    return out
